# revision 1
# baseline (speedup 1.0000x reference)
"""AdaWinBlock1d Trainium2 kernel (8 NeuronCores, data-parallel over batch).

Per-core: 2 samples. Per sample:
  style pooling -> fc1/fc2 -> (1+gamma), beta per channel
  norm1(x) + lrelu -> conv1(k=3, reflect) -> norm2 + lrelu -> conv2(k=3, reflect)
  out = (conv2 + conv1x1_shortcut(x)) / sqrt(2)

Window (37) sums computed with a single DVE tensor_tensor_scan recurrence
  W[l] = (x[l+18] + W[l-1]) - x[l-19]  over zero-padded buffers.
Convs are bf16 TensorEngine matmuls accumulated in PSUM (taps = shifted views
of the reflect-padded activations). Host precomputes mask/(den+eps) ("im"),
1/len, transposed bf16 weights, and packed per-channel bias columns.
"""
import math
import os
import sys
import types

import numpy as np
import ml_dtypes

B, NCORES = 16, 8
BPC = B // NCORES          # samples per core
CIN, COUT, SD, L = 256, 512, 128, 4096
WIN, HWIN = 37, 18
PADL, PADR = 37, 18        # scan needs x[l-37..l+18] around each output row
LP = PADL + L + PADR       # padded length
LS = L + PADR              # scan output length (l = -18 .. L-1)
TS = 512                   # psum free-dim tile
NT = L // TS
EPS = 1e-9
ISQ2 = 1.0 / math.sqrt(2.0)
CCIN = CIN // 128          # 2 input-channel chunks
CCOUT = COUT // 128        # 4 output-channel chunks

BF16 = ml_dtypes.bfloat16

_PROG = None
LAST_EXEC_TIME_NS = None


def _install_ntff_hook():
    if 'antenv.axon_hooks' in sys.modules:
        return
    try:
        from trn_agent_boot.trn_boot import _ntff_profile_via_ctypes
        hook = _ntff_profile_via_ctypes('/opt/axon/libaxon_pjrt.so')
    except Exception:
        hook = None
    mod = types.ModuleType('antenv.axon_hooks')
    mod.get_axon_ntff_profile_hook = lambda: hook
    mod.set_axon_ntff_profile_hook = lambda h: None
    sys.modules['antenv.axon_hooks'] = mod


def _chunk_norm(nc, tc, pools, src, im_t, eps_ap, scale_t, beta_t, dst_ap):
    """One 128-row chunk of the masked sliding-window instance norm + affine +
    leaky relu. src: padded (128, LP) bf16 input tile (zero pads). dst_ap:
    (128, L) bf16 output AP (interior of a reflect-padded conv input)."""
    import concourse.mybir as mybir
    AL = mybir.AluOpType
    ACT = mybir.ActivationFunctionType
    BF = mybir.dt.bfloat16

    ws = pools["ws"].tile([128, LS], BF, tag="ws")
    # W[l] = (src[37+l+18] + W[l-1]) - src[l+18-37+...]: window sum, l=-18..L-1
    nc.vector.tensor_tensor_scan(
        out=ws[:, 0:LS], data0=src[:, PADL:PADL + LS], data1=src[:, 0:LS],
        initial=0.0, op0=AL.add, op1=AL.subtract)
    wsv = ws[:, PADR:PADR + L]
    # mean = W * im   (im = mask/(den+eps), broadcast rows)
    nc.vector.tensor_tensor(out=wsv, in0=wsv, in1=im_t[:], op=AL.mult)
    # t = x - mean
    tt = pools["tt"].tile([128, L], BF, tag="tt")
    nc.vector.tensor_tensor(out=tt[:], in0=src[:, PADL:PADL + L], in1=wsv,
                            op=AL.subtract)
    # g = (mask * t)^2  into padded buffer
    gp = pools["gp"].tile([128, LP], BF, tag="gp")
    nc.gpsimd.memset(gp[:, 0:PADL], 0.0)
    nc.gpsimd.memset(gp[:, PADL + L:], 0.0)
    nc.vector.scalar_tensor_tensor(
        out=gp[:, PADL:PADL + L], in0=im_t[:], scalar=0.0, in1=tt[:],
        op0=AL.is_gt, op1=AL.mult)
    nc.scalar.activation(gp[:, PADL:PADL + L], gp[:, PADL:PADL + L], ACT.Square)
    # wsg = window sum of g (reuse ws tile)
    nc.vector.tensor_tensor_scan(
        out=ws[:, 0:LS], data0=gp[:, PADL:PADL + LS], data1=gp[:, 0:LS],
        initial=0.0, op0=AL.add, op1=AL.subtract)
    # v = max(wsg, 0) * im ; rstd = 1/sqrt(v + eps)
    nc.vector.scalar_tensor_tensor(
        out=wsv, in0=wsv, scalar=0.0, in1=im_t[:], op0=AL.max, op1=AL.mult)
    nc.scalar.activation(wsv, wsv, ACT.Abs_reciprocal_sqrt,
                         bias=eps_ap, scale=1.0)
    # xn = t * rstd ; y = prelu(xn * (1+gamma) + beta, 0.2)
    nc.vector.tensor_tensor(out=tt[:], in0=tt[:], in1=wsv, op=AL.mult)
    nc.scalar.activation(dst_ap, tt[:], ACT.Prelu,
                         bias=beta_t[:], scale=scale_t[:], alpha=0.2)


def _build_program():
    import concourse.tile as tile
    from concourse import bacc, mybir
    F32 = mybir.dt.float32
    BF = mybir.dt.bfloat16
    AL = mybir.AluOpType
    ACT = mybir.ActivationFunctionType

    nc = bacc.Bacc("TRN2", target_bir_lowering=False, debug=False)

    d_x = nc.dram_tensor("x", [BPC, CIN, L], BF, kind="ExternalInput")
    d_s = nc.dram_tensor("s", [BPC, SD, L], BF, kind="ExternalInput")
    d_im = nc.dram_tensor("im", [BPC, L], BF, kind="ExternalInput")
    d_ivl = nc.dram_tensor("invlen", [BPC, 128], F32, kind="ExternalInput")
    d_w1 = nc.dram_tensor("w1t", [128, 3 * CCIN * COUT], BF, kind="ExternalInput")
    d_w2 = nc.dram_tensor("w2t", [128, 3 * CCOUT * COUT], BF, kind="ExternalInput")
    d_sc = nc.dram_tensor("sct", [128, CCIN * COUT], BF, kind="ExternalInput")
    d_f1 = nc.dram_tensor("fc1t", [128, 2 * CIN], F32, kind="ExternalInput")
    d_f2 = nc.dram_tensor("fc2t", [128, 2 * COUT], F32, kind="ExternalInput")
    # packed per-partition constant columns:
    # 0:4 conv1_b | 4:8 conv2_b/sqrt2 | 8:10 1+fc1_b[:256] | 10:12 fc1_b[256:]
    # 12:16 1+fc2_b[:512] | 16:20 fc2_b[512:] | 20 eps
    d_pc = nc.dram_tensor("pcons", [128, 21], F32, kind="ExternalInput")
    d_out = nc.dram_tensor("out", [BPC, COUT, L], F32, kind="ExternalOutput")

    with tile.TileContext(nc) as tc:
        import contextlib
        with contextlib.ExitStack() as ctx:
            pw = ctx.enter_context(tc.tile_pool(name="weights", bufs=1))
            psty = ctx.enter_context(tc.tile_pool(name="spool", bufs=1))
            pim = ctx.enter_context(tc.tile_pool(name="impool", bufs=1))
            pxp = ctx.enter_context(tc.tile_pool(name="xpool", bufs=2))
            pap = ctx.enter_context(tc.tile_pool(name="apool", bufs=5))
            php = ctx.enter_context(tc.tile_pool(name="hpool", bufs=4))
            pws = ctx.enter_context(tc.tile_pool(name="wspool", bufs=2))
            ptt = ctx.enter_context(tc.tile_pool(name="ttpool", bufs=2))
            pgp = ctx.enter_context(tc.tile_pool(name="gppool", bufs=2))
            pot = ctx.enter_context(tc.tile_pool(name="opool", bufs=2))
            psm = ctx.enter_context(tc.tile_pool(name="small", bufs=28))
            ppc = ctx.enter_context(tc.tile_pool(name="psc", bufs=4, space="PSUM"))
            pps = ctx.enter_context(tc.tile_pool(name="pss", bufs=2, space="PSUM"))
            pools = {"ws": pws, "tt": ptt, "gp": pgp}

            # persistent weights
            w1_t = pw.tile([128, 3 * CCIN * COUT], BF)
            nc.sync.dma_start(w1_t[:], d_w1.ap())
            w2_t = pw.tile([128, 3 * CCOUT * COUT], BF)
            nc.sync.dma_start(w2_t[:], d_w2.ap())
            sc_t = pw.tile([128, CCIN * COUT], BF)
            nc.sync.dma_start(sc_t[:], d_sc.ap())
            f1_t = pw.tile([128, 2 * CIN], F32)
            nc.sync.dma_start(f1_t[:], d_f1.ap())
            f2_t = pw.tile([128, 2 * COUT], F32)
            nc.sync.dma_start(f2_t[:], d_f2.ap())
            pc_t = pw.tile([128, 21], F32)
            nc.sync.dma_start(pc_t[:], d_pc.ap())
            eps_ap = pc_t[:, 20:21]

            for b in range(BPC):
                # ---- per-sample inputs
                im_t = pim.tile([128, L], BF, tag="im")
                nc.sync.dma_start(im_t[0:1, :], d_im.ap()[b:b + 1, :])
                nc.gpsimd.partition_broadcast(im_t[:, :], im_t[0:1, :])
                ivl_t = psm.tile([128, 1], F32, tag="ivl")
                nc.sync.dma_start(ivl_t[:], d_ivl.ap()[b].unsqueeze(1))
                s_t = psty.tile([128, L], BF, tag="s")
                nc.sync.dma_start(s_t[:], d_s.ap()[b])

                # ---- style pooling: sp = sum(s*mask)/len
                nc.vector.scalar_tensor_tensor(
                    out=s_t[:], in0=im_t[:], scalar=0.0, in1=s_t[:],
                    op0=AL.is_gt, op1=AL.mult)
                sp_t = psm.tile([128, 1], F32, tag="sp")
                nc.scalar.activation(s_t[:], s_t[:], ACT.Copy, bias=0.0,
                                     scale=ivl_t[:], accum_out=sp_t[:])

                # ---- fc1 / fc2 -> scale (1+gamma) and beta chunks
                def fc_chunks(fc_t, nch, g_off, b_off):
                    scales, betas = [], []
                    for j in range(2 * nch):
                        pst = pps.tile([128, 1], F32, tag="stps")
                        nc.tensor.matmul(pst[:], fc_t[:, j * 128:(j + 1) * 128],
                                         sp_t[:], start=True, stop=True)
                        dst = psm.tile([128, 1], F32, tag="sb")
                        off = (g_off + j) if j < nch else (b_off + j - nch)
                        nc.vector.tensor_tensor(
                            out=dst[:], in0=pst[:], in1=pc_t[:, off:off + 1],
                            op=AL.add)
                        (scales if j < nch else betas).append(dst)
                    return scales, betas

                sc1, be1 = fc_chunks(f1_t, CCIN, 8, 10 - CCIN)
                sc2, be2 = fc_chunks(f2_t, CCOUT, 12, 16 - CCOUT)

                # ---- norm1 + lrelu -> a1 (reflect-padded bf16)
                a1 = []
                xp = []
                for cc in range(CCIN):
                    xpt = pxp.tile([128, LP], BF, tag="xp")
                    nc.gpsimd.memset(xpt[:, 0:PADL], 0.0)
                    nc.gpsimd.memset(xpt[:, PADL + L:], 0.0)
                    nc.sync.dma_start(xpt[:, PADL:PADL + L],
                                      d_x.ap()[b, cc * 128:(cc + 1) * 128, :])
                    xp.append(xpt)
                    a1t = pap.tile([128, L + 2], BF, tag="ap")
                    _chunk_norm(nc, tc, pools, xpt, im_t, eps_ap,
                                sc1[cc], be1[cc], a1t[:, 1:1 + L])
                    nc.gpsimd.tensor_copy(a1t[:, 0:1], a1t[:, 2:3])
                    nc.gpsimd.tensor_copy(a1t[:, L + 1:L + 2], a1t[:, L - 1:L])
                    a1.append(a1t)

                # ---- conv1 -> h (padded bf16) with bias
                hp = []
                for m in range(CCOUT):
                    ht = php.tile([128, LP], BF, tag="hp")
                    nc.gpsimd.memset(ht[:, 0:PADL], 0.0)
                    nc.gpsimd.memset(ht[:, PADL + L:], 0.0)
                    hp.append(ht)
                for m in range(CCOUT):
                    for nt in range(NT):
                        ps = ppc.tile([128, TS], F32, tag="cps")
                        k = 0
                        for tap in range(3):
                            for cc in range(CCIN):
                                nc.tensor.matmul(
                                    ps[:],
                                    w1_t[:, (tap * CCIN + cc) * COUT + m * 128:
                                         (tap * CCIN + cc) * COUT + (m + 1) * 128],
                                    a1[cc][:, tap + nt * TS:tap + nt * TS + TS],
                                    start=(k == 0), stop=(k == 3 * CCIN - 1))
                                k += 1
                        nc.scalar.activation(
                            hp[m][:, PADL + nt * TS:PADL + (nt + 1) * TS], ps[:],
                            ACT.Identity, bias=pc_t[:, m:m + 1], scale=1.0)

                # ---- norm2 + lrelu -> a2
                a2 = []
                for mc in range(CCOUT):
                    a2t = pap.tile([128, L + 2], BF, tag="ap")
                    _chunk_norm(nc, tc, pools, hp[mc], im_t, eps_ap,
                                sc2[mc], be2[mc], a2t[:, 1:1 + L])
                    nc.gpsimd.tensor_copy(a2t[:, 0:1], a2t[:, 2:3])
                    nc.gpsimd.tensor_copy(a2t[:, L + 1:L + 2], a2t[:, L - 1:L])
                    a2.append(a2t)

                # ---- conv2 + shortcut -> out
                for m in range(CCOUT):
                    for nt in range(NT):
                        ps = ppc.tile([128, TS], F32, tag="cps")
                        k = 0
                        nmm = 3 * CCOUT + CCIN
                        for tap in range(3):
                            for cc in range(CCOUT):
                                nc.tensor.matmul(
                                    ps[:],
                                    w2_t[:, (tap * CCOUT + cc) * COUT + m * 128:
                                         (tap * CCOUT + cc) * COUT + (m + 1) * 128],
                                    a2[cc][:, tap + nt * TS:tap + nt * TS + TS],
                                    start=(k == 0), stop=False)
                                k += 1
                        for cc in range(CCIN):
                            nc.tensor.matmul(
                                ps[:],
                                sc_t[:, cc * COUT + m * 128:cc * COUT + (m + 1) * 128],
                                xp[cc][:, PADL + nt * TS:PADL + nt * TS + TS],
                                start=False, stop=(k == nmm - 1))
                            k += 1
                        ot = pot.tile([128, TS], F32, tag="ot")
                        nc.vector.tensor_scalar(
                            out=ot[:], in0=ps[:], scalar1=ISQ2,
                            scalar2=pc_t[:, 4 + m:5 + m], op0=AL.mult, op1=AL.add)
                        nc.sync.dma_start(
                            d_out.ap()[b, m * 128:(m + 1) * 128,
                                       nt * TS:(nt + 1) * TS], ot[:])

    nc.compile()
    return nc


def _host_prep(x, s, lengths, fc1_w, fc1_b, fc2_w, fc2_b,
               conv1_w, conv1_b, conv2_w, conv2_b, sc_w):
    f32 = np.float32
    lengths = np.asarray(lengths).astype(np.int64)
    mask = (np.arange(L)[None, :] < lengths[:, None]).astype(f32)
    c = np.concatenate([np.zeros((B, 1), f32),
                        np.cumsum(mask, axis=1, dtype=f32)], axis=1)
    hi = np.clip(np.arange(L) + HWIN + 1, 0, L)
    lo = np.clip(np.arange(L) - HWIN, 0, L)
    den = c[:, hi] - c[:, lo]
    im = (mask / (den + f32(EPS))).astype(BF16)
    invlen = np.repeat((1.0 / lengths.astype(f32))[:, None], 128, axis=1).astype(f32)

    def conv_t(w, ncc):  # (COUT, ncc*128, 3) -> (128, 3*ncc*COUT) [tap][cc]
        co, ci, kk = w.shape
        a = np.transpose(w, (2, 1, 0)).reshape(kk, ncc, 128, co)
        return np.ascontiguousarray(
            np.transpose(a, (2, 0, 1, 3)).reshape(128, kk * ncc * co)).astype(BF16)

    w1t = conv_t(np.asarray(conv1_w, f32), CCIN)
    w2t = conv_t(np.asarray(conv2_w, f32), CCOUT)
    sct = conv_t(np.asarray(sc_w, f32), CCIN)
    fc1t = np.ascontiguousarray(np.asarray(fc1_w, f32).T)
    fc2t = np.ascontiguousarray(np.asarray(fc2_w, f32).T)

    pc = np.zeros((128, 21), f32)
    pc[:, 0:4] = np.asarray(conv1_b, f32).reshape(4, 128).T
    pc[:, 4:8] = (np.asarray(conv2_b, f32) * ISQ2).reshape(4, 128).T
    pc[:, 8:10] = (1.0 + np.asarray(fc1_b, f32)[:CIN]).reshape(2, 128).T
    pc[:, 10:12] = np.asarray(fc1_b, f32)[CIN:].reshape(2, 128).T
    pc[:, 12:16] = (1.0 + np.asarray(fc2_b, f32)[:COUT]).reshape(4, 128).T
    pc[:, 16:20] = np.asarray(fc2_b, f32)[COUT:].reshape(4, 128).T
    pc[:, 20] = f32(EPS)

    xb = np.asarray(x, f32).astype(BF16)
    sb = np.asarray(s, f32).astype(BF16)
    shared = dict(w1t=w1t, w2t=w2t, sct=sct, fc1t=fc1t, fc2t=fc2t, pcons=pc)
    in_maps = []
    for cidx in range(NCORES):
        b0 = cidx * BPC
        m = dict(shared)
        m["x"] = np.ascontiguousarray(xb[b0:b0 + BPC])
        m["s"] = np.ascontiguousarray(sb[b0:b0 + BPC])
        m["im"] = np.ascontiguousarray(im[b0:b0 + BPC])
        m["invlen"] = np.ascontiguousarray(invlen[b0:b0 + BPC])
        in_maps.append(m)
    return in_maps


def kernel(**inputs):
    global _PROG, LAST_EXEC_TIME_NS
    _install_ntff_hook()
    from concourse.bass_utils import run_bass_kernel_spmd

    in_maps = _host_prep(**inputs)
    if _PROG is None:
        _PROG = _build_program()
    trace = bool(os.environ.get("AWB_TRACE"))
    res = run_bass_kernel_spmd(_PROG, in_maps, core_ids=list(range(NCORES)),
                               trace=trace)
    LAST_EXEC_TIME_NS = res.exec_time_ns
    out = np.concatenate([res.results[c]["out"] for c in range(NCORES)], axis=0)
    return np.ascontiguousarray(out.astype(np.float32))


# revision 3
# speedup vs baseline: 1.0492x; 1.0492x over previous
"""AdaWinBlock1d Trainium2 kernel (8 NeuronCores, data-parallel over batch).

Per-core: 2 samples. Per sample:
  style pooling -> fc1/fc2 -> (1+gamma), beta per channel
  norm1(x) + lrelu -> conv1(k=3, reflect) -> norm2 + lrelu -> conv2(k=3, reflect)
  out = (conv2 + conv1x1_shortcut(x)) / sqrt(2)

Window (37) sums computed with a single DVE tensor_tensor_scan recurrence
  W[l] = (x[l+18] + W[l-1]) - x[l-19]  over zero-padded buffers.
Convs are bf16 TensorEngine matmuls accumulated in PSUM (taps = shifted views
of the reflect-padded activations). Host precomputes mask/(den+eps) ("im"),
1/len, transposed bf16 weights, and packed per-channel bias columns.
"""
import math
import os
import sys
import types

import numpy as np
import ml_dtypes

B, NCORES = 16, 8
BPC = B // NCORES          # samples per core
CIN, COUT, SD, L = 256, 512, 128, 4096
WIN, HWIN = 37, 18
PADL, PADR = 37, 18        # scan needs x[l-37..l+18] around each output row
LP = PADL + L + PADR       # padded length
LS = L + PADR              # scan output length (l = -18 .. L-1)
TS = 512                   # psum free-dim tile
NT = L // TS
EPS = 1e-9
ISQ2 = 1.0 / math.sqrt(2.0)
CCIN = CIN // 128          # 2 input-channel chunks
CCOUT = COUT // 128        # 4 output-channel chunks

BF16 = ml_dtypes.bfloat16

_PROG = None
LAST_EXEC_TIME_NS = None


def _install_ntff_hook():
    if 'antenv.axon_hooks' in sys.modules:
        return
    try:
        from trn_agent_boot.trn_boot import _ntff_profile_via_ctypes
        hook = _ntff_profile_via_ctypes('/opt/axon/libaxon_pjrt.so')
    except Exception:
        hook = None
    mod = types.ModuleType('antenv.axon_hooks')
    mod.get_axon_ntff_profile_hook = lambda: hook
    mod.set_axon_ntff_profile_hook = lambda h: None
    sys.modules['antenv.axon_hooks'] = mod


def _chunk_norm(nc, tc, pools, src, im_t, mk_t, eps_ap, scale_t, beta_t, dst_ap):
    """One 128-row chunk of the masked sliding-window instance norm + affine +
    leaky relu. src: padded (128, LP) bf16 input tile (zero pads). dst_ap:
    (128, L) bf16 output AP (interior of a reflect-padded conv input)."""
    import concourse.mybir as mybir
    AL = mybir.AluOpType
    ACT = mybir.ActivationFunctionType
    BF = mybir.dt.bfloat16

    ws = pools["ws"].tile([128, LS], BF, tag="ws")
    # W[l] = (src[37+l+18] + W[l-1]) - src[l+18-37+...]: window sum, l=-18..L-1
    nc.vector.tensor_tensor_scan(
        out=ws[:, 0:LS], data0=src[:, PADL:PADL + LS], data1=src[:, 0:LS],
        initial=0.0, op0=AL.add, op1=AL.subtract)
    wsv = ws[:, PADR:PADR + L]
    # mean = W * im   (im = mask/(den+eps), broadcast rows)
    nc.vector.tensor_tensor(out=wsv, in0=wsv, in1=im_t[:], op=AL.mult)
    # t = x - mean
    tt = pools["tt"].tile([128, L], BF, tag="tt")
    nc.vector.tensor_tensor(out=tt[:], in0=src[:, PADL:PADL + L], in1=wsv,
                            op=AL.subtract)
    # g = (mask * t)^2  into padded buffer
    gp = pools["gp"].tile([128, LP], BF, tag="gp")
    nc.gpsimd.memset(gp[:, 0:PADL], 0.0)
    nc.gpsimd.memset(gp[:, PADL + L:], 0.0)
    nc.vector.tensor_tensor(out=gp[:, PADL:PADL + L], in0=tt[:], in1=mk_t[:],
                            op=AL.mult)
    nc.scalar.activation(gp[:, PADL:PADL + L], gp[:, PADL:PADL + L], ACT.Square)
    # wsg = window sum of g (reuse ws tile)
    nc.vector.tensor_tensor_scan(
        out=ws[:, 0:LS], data0=gp[:, PADL:PADL + LS], data1=gp[:, 0:LS],
        initial=0.0, op0=AL.add, op1=AL.subtract)
    # v = wsg * im (Abs_reciprocal_sqrt's |.| guards tiny-negative drift)
    nc.vector.tensor_tensor(out=wsv, in0=wsv, in1=im_t[:], op=AL.mult)
    nc.scalar.activation(wsv, wsv, ACT.Abs_reciprocal_sqrt,
                         bias=eps_ap, scale=1.0)
    # xn = t * rstd ; y = prelu(xn * (1+gamma) + beta, 0.2)
    nc.vector.tensor_tensor(out=tt[:], in0=tt[:], in1=wsv, op=AL.mult)
    nc.scalar.activation(dst_ap, tt[:], ACT.Prelu,
                         bias=beta_t[:], scale=scale_t[:], alpha=0.2)


def _build_program():
    import concourse.tile as tile
    from concourse import bacc, mybir
    F32 = mybir.dt.float32
    BF = mybir.dt.bfloat16
    AL = mybir.AluOpType
    ACT = mybir.ActivationFunctionType

    nc = bacc.Bacc("TRN2", target_bir_lowering=False, debug=False)

    d_x = nc.dram_tensor("x", [BPC, CIN, L], BF, kind="ExternalInput")
    d_s = nc.dram_tensor("s", [BPC, SD, L], BF, kind="ExternalInput")
    d_im = nc.dram_tensor("im", [BPC, L], BF, kind="ExternalInput")
    d_mk = nc.dram_tensor("mk", [BPC, L], BF, kind="ExternalInput")
    d_ivl = nc.dram_tensor("invlen", [BPC, 128], F32, kind="ExternalInput")
    d_w1 = nc.dram_tensor("w1t", [128, 3 * CCIN * COUT], BF, kind="ExternalInput")
    d_w2 = nc.dram_tensor("w2t", [128, 3 * CCOUT * COUT], BF, kind="ExternalInput")
    d_sc = nc.dram_tensor("sct", [128, CCIN * COUT], BF, kind="ExternalInput")
    d_f1 = nc.dram_tensor("fc1t", [128, 2 * CIN], F32, kind="ExternalInput")
    d_f2 = nc.dram_tensor("fc2t", [128, 2 * COUT], F32, kind="ExternalInput")
    # packed per-partition constant columns:
    # 0:4 conv1_b | 4:8 conv2_b/sqrt2 | 8:10 1+fc1_b[:256] | 10:12 fc1_b[256:]
    # 12:16 1+fc2_b[:512] | 16:20 fc2_b[512:] | 20 eps
    d_pc = nc.dram_tensor("pcons", [128, 21], F32, kind="ExternalInput")
    d_out = nc.dram_tensor("out", [BPC, COUT, L], F32, kind="ExternalOutput")

    with tile.TileContext(nc) as tc:
        import contextlib
        with contextlib.ExitStack() as ctx:
            pw = ctx.enter_context(tc.tile_pool(name="weights", bufs=1))
            psty = ctx.enter_context(tc.tile_pool(name="spool", bufs=1))
            pim = ctx.enter_context(tc.tile_pool(name="impool", bufs=1))
            pxp = ctx.enter_context(tc.tile_pool(name="xpool", bufs=2))
            pap = ctx.enter_context(tc.tile_pool(name="apool", bufs=5))
            php = ctx.enter_context(tc.tile_pool(name="hpool", bufs=4))
            pws = ctx.enter_context(tc.tile_pool(name="wspool", bufs=2))
            ptt = ctx.enter_context(tc.tile_pool(name="ttpool", bufs=2))
            pgp = ctx.enter_context(tc.tile_pool(name="gppool", bufs=2))
            pot = ctx.enter_context(tc.tile_pool(name="opool", bufs=2))
            psm = ctx.enter_context(tc.tile_pool(name="small", bufs=28))
            ppc = ctx.enter_context(tc.tile_pool(name="psc", bufs=6, space="PSUM"))
            pps = ctx.enter_context(tc.tile_pool(name="pss", bufs=2, space="PSUM"))
            pools = {"ws": pws, "tt": ptt, "gp": pgp}

            # persistent weights
            w1_t = pw.tile([128, 3 * CCIN * COUT], BF)
            nc.sync.dma_start(w1_t[:], d_w1.ap())
            w2_t = pw.tile([128, 3 * CCOUT * COUT], BF)
            nc.sync.dma_start(w2_t[:], d_w2.ap())
            sc_t = pw.tile([128, CCIN * COUT], BF)
            nc.sync.dma_start(sc_t[:], d_sc.ap())
            f1_t = pw.tile([128, 2 * CIN], F32)
            nc.sync.dma_start(f1_t[:], d_f1.ap())
            f2_t = pw.tile([128, 2 * COUT], F32)
            nc.sync.dma_start(f2_t[:], d_f2.ap())
            pc_t = pw.tile([128, 21], F32)
            nc.sync.dma_start(pc_t[:], d_pc.ap())
            eps_ap = pc_t[:, 20:21]

            for b in range(BPC):
                # ---- per-sample inputs
                im_t = pim.tile([128, L], BF, tag="im")
                nc.sync.dma_start(im_t[0:1, :], d_im.ap()[b:b + 1, :])
                nc.gpsimd.partition_broadcast(im_t[:, :], im_t[0:1, :])
                mk_t = pim.tile([128, L], BF, tag="mk")
                nc.sync.dma_start(mk_t[0:1, :], d_mk.ap()[b:b + 1, :])
                nc.gpsimd.partition_broadcast(mk_t[:, :], mk_t[0:1, :])
                ivl_t = psm.tile([128, 1], F32, tag="ivl")
                nc.sync.dma_start(ivl_t[:], d_ivl.ap()[b].unsqueeze(1))
                s_t = psty.tile([128, L], BF, tag="s")
                nc.sync.dma_start(s_t[:], d_s.ap()[b])

                # ---- style pooling: sp = sum(s*mask)/len
                nc.vector.tensor_tensor(out=s_t[:], in0=s_t[:], in1=mk_t[:],
                                        op=AL.mult)
                sp_t = psm.tile([128, 1], F32, tag="sp")
                nc.scalar.activation(s_t[:], s_t[:], ACT.Copy, bias=0.0,
                                     scale=ivl_t[:], accum_out=sp_t[:])

                # ---- fc1 / fc2 -> scale (1+gamma) and beta chunks
                def fc_chunks(fc_t, nch, g_off, b_off):
                    scales, betas = [], []
                    for j in range(2 * nch):
                        pst = pps.tile([128, 1], F32, tag="stps")
                        nc.tensor.matmul(pst[:], fc_t[:, j * 128:(j + 1) * 128],
                                         sp_t[:], start=True, stop=True)
                        dst = psm.tile([128, 1], F32, tag="sb")
                        off = (g_off + j) if j < nch else (b_off + j - nch)
                        nc.vector.tensor_tensor(
                            out=dst[:], in0=pst[:], in1=pc_t[:, off:off + 1],
                            op=AL.add)
                        (scales if j < nch else betas).append(dst)
                    return scales, betas

                sc1, be1 = fc_chunks(f1_t, CCIN, 8, 10 - CCIN)
                sc2, be2 = fc_chunks(f2_t, CCOUT, 12, 16 - CCOUT)

                # ---- norm1 + lrelu -> a1 (reflect-padded bf16)
                a1 = []
                xp = []
                for cc in range(CCIN):
                    xpt = pxp.tile([128, LP], BF, tag="xp")
                    nc.gpsimd.memset(xpt[:, 0:PADL], 0.0)
                    nc.gpsimd.memset(xpt[:, PADL + L:], 0.0)
                    nc.sync.dma_start(xpt[:, PADL:PADL + L],
                                      d_x.ap()[b, cc * 128:(cc + 1) * 128, :])
                    xp.append(xpt)
                    a1t = pap.tile([128, L + 2], BF, tag="ap")
                    _chunk_norm(nc, tc, pools, xpt, im_t, mk_t, eps_ap,
                                sc1[cc], be1[cc], a1t[:, 1:1 + L])
                    nc.gpsimd.tensor_copy(a1t[:, 0:1], a1t[:, 2:3])
                    nc.gpsimd.tensor_copy(a1t[:, L + 1:L + 2], a1t[:, L - 1:L])
                    a1.append(a1t)

                # ---- conv1 -> h (padded bf16) with bias
                hp = []
                for m in range(CCOUT):
                    ht = php.tile([128, LP], BF, tag="hp")
                    nc.gpsimd.memset(ht[:, 0:PADL], 0.0)
                    nc.gpsimd.memset(ht[:, PADL + L:], 0.0)
                    hp.append(ht)
                NTG = 4
                for m in range(CCOUT):
                    for g in range(NT // NTG):
                        pss = [ppc.tile([128, TS], F32, tag="cps",
                                        name=f"c1ps_{m}_{g}_{j}")
                               for j in range(NTG)]
                        nw = 3 * CCIN
                        for k, (tap, cc) in enumerate(
                                (t, c) for t in range(3) for c in range(CCIN)):
                            lhs = w1_t[:, (tap * CCIN + cc) * COUT + m * 128:
                                       (tap * CCIN + cc) * COUT + (m + 1) * 128]
                            for j in range(NTG):
                                nt = g * NTG + j
                                nc.tensor.matmul(
                                    pss[j][:], lhs,
                                    a1[cc][:, tap + nt * TS:tap + nt * TS + TS],
                                    start=(k == 0), stop=(k == nw - 1))
                        for j in range(NTG):
                            nt = g * NTG + j
                            nc.scalar.activation(
                                hp[m][:, PADL + nt * TS:PADL + (nt + 1) * TS],
                                pss[j][:], ACT.Identity,
                                bias=pc_t[:, m:m + 1], scale=1.0)

                # ---- norm2 + lrelu -> a2
                a2 = []
                for mc in range(CCOUT):
                    a2t = pap.tile([128, L + 2], BF, tag="ap")
                    _chunk_norm(nc, tc, pools, hp[mc], im_t, mk_t, eps_ap,
                                sc2[mc], be2[mc], a2t[:, 1:1 + L])
                    nc.gpsimd.tensor_copy(a2t[:, 0:1], a2t[:, 2:3])
                    nc.gpsimd.tensor_copy(a2t[:, L + 1:L + 2], a2t[:, L - 1:L])
                    a2.append(a2t)

                # ---- conv2 + shortcut -> out
                for m in range(CCOUT):
                    for g in range(NT // NTG):
                        pss = [ppc.tile([128, TS], F32, tag="cps",
                                        name=f"c2ps_{m}_{g}_{j}")
                               for j in range(NTG)]
                        nw = 3 * CCOUT + CCIN
                        wlist = [(t, c, False) for t in range(3)
                                 for c in range(CCOUT)]
                        wlist += [(0, c, True) for c in range(CCIN)]
                        for k, (tap, cc, is_sc) in enumerate(wlist):
                            if is_sc:
                                lhs = sc_t[:, cc * COUT + m * 128:
                                           cc * COUT + (m + 1) * 128]
                            else:
                                lhs = w2_t[:, (tap * CCOUT + cc) * COUT + m * 128:
                                           (tap * CCOUT + cc) * COUT + (m + 1) * 128]
                            for j in range(NTG):
                                nt = g * NTG + j
                                if is_sc:
                                    rhs = xp[cc][:, PADL + nt * TS:
                                                 PADL + nt * TS + TS]
                                else:
                                    rhs = a2[cc][:, tap + nt * TS:
                                                 tap + nt * TS + TS]
                                nc.tensor.matmul(pss[j][:], lhs, rhs,
                                                 start=(k == 0),
                                                 stop=(k == nw - 1))
                        for j in range(NTG):
                            nt = g * NTG + j
                            ot = pot.tile([128, TS], F32, tag="ot")
                            nc.scalar.activation(
                                ot[:], pss[j][:], ACT.Identity,
                                bias=pc_t[:, 4 + m:5 + m], scale=ISQ2)
                            nc.sync.dma_start(
                                d_out.ap()[b, m * 128:(m + 1) * 128,
                                           nt * TS:(nt + 1) * TS], ot[:])

    nc.compile()
    return nc


def _host_prep(x, s, lengths, fc1_w, fc1_b, fc2_w, fc2_b,
               conv1_w, conv1_b, conv2_w, conv2_b, sc_w):
    f32 = np.float32
    lengths = np.asarray(lengths).astype(np.int64)
    mask = (np.arange(L)[None, :] < lengths[:, None]).astype(f32)
    c = np.concatenate([np.zeros((B, 1), f32),
                        np.cumsum(mask, axis=1, dtype=f32)], axis=1)
    hi = np.clip(np.arange(L) + HWIN + 1, 0, L)
    lo = np.clip(np.arange(L) - HWIN, 0, L)
    den = c[:, hi] - c[:, lo]
    im = (mask / (den + f32(EPS))).astype(BF16)
    maskb = mask.astype(BF16)
    invlen = np.repeat((1.0 / lengths.astype(f32))[:, None], 128, axis=1).astype(f32)

    def conv_t(w, ncc):  # (COUT, ncc*128, 3) -> (128, 3*ncc*COUT) [tap][cc]
        co, ci, kk = w.shape
        a = np.transpose(w, (2, 1, 0)).reshape(kk, ncc, 128, co)
        return np.ascontiguousarray(
            np.transpose(a, (2, 0, 1, 3)).reshape(128, kk * ncc * co)).astype(BF16)

    w1t = conv_t(np.asarray(conv1_w, f32), CCIN)
    w2t = conv_t(np.asarray(conv2_w, f32), CCOUT)
    sct = conv_t(np.asarray(sc_w, f32), CCIN)
    fc1t = np.ascontiguousarray(np.asarray(fc1_w, f32).T)
    fc2t = np.ascontiguousarray(np.asarray(fc2_w, f32).T)

    pc = np.zeros((128, 21), f32)
    pc[:, 0:4] = np.asarray(conv1_b, f32).reshape(4, 128).T
    pc[:, 4:8] = (np.asarray(conv2_b, f32) * ISQ2).reshape(4, 128).T
    pc[:, 8:10] = (1.0 + np.asarray(fc1_b, f32)[:CIN]).reshape(2, 128).T
    pc[:, 10:12] = np.asarray(fc1_b, f32)[CIN:].reshape(2, 128).T
    pc[:, 12:16] = (1.0 + np.asarray(fc2_b, f32)[:COUT]).reshape(4, 128).T
    pc[:, 16:20] = np.asarray(fc2_b, f32)[COUT:].reshape(4, 128).T
    pc[:, 20] = f32(EPS)

    xb = np.asarray(x, f32).astype(BF16)
    sb = np.asarray(s, f32).astype(BF16)
    shared = dict(w1t=w1t, w2t=w2t, sct=sct, fc1t=fc1t, fc2t=fc2t, pcons=pc)
    in_maps = []
    for cidx in range(NCORES):
        b0 = cidx * BPC
        m = dict(shared)
        m["x"] = np.ascontiguousarray(xb[b0:b0 + BPC])
        m["s"] = np.ascontiguousarray(sb[b0:b0 + BPC])
        m["im"] = np.ascontiguousarray(im[b0:b0 + BPC])
        m["mk"] = np.ascontiguousarray(maskb[b0:b0 + BPC])
        m["invlen"] = np.ascontiguousarray(invlen[b0:b0 + BPC])
        in_maps.append(m)
    return in_maps


def kernel(**inputs):
    global _PROG, LAST_EXEC_TIME_NS
    _install_ntff_hook()
    from concourse.bass_utils import run_bass_kernel_spmd

    in_maps = _host_prep(**inputs)
    if _PROG is None:
        _PROG = _build_program()
    trace = bool(os.environ.get("AWB_TRACE"))
    res = run_bass_kernel_spmd(_PROG, in_maps, core_ids=list(range(NCORES)),
                               trace=trace)
    LAST_EXEC_TIME_NS = res.exec_time_ns
    out = np.concatenate([res.results[c]["out"] for c in range(NCORES)], axis=0)
    return np.ascontiguousarray(out.astype(np.float32))
